# revision 8
# baseline (speedup 1.0000x reference)
"""Trainium2 Bass kernel for nn_MergerSingleW (vq_codebook).

Reference math:
    alpha = softplus(alpha_raw[0]) + 1e-6
    Wq    = nearest level in alpha*{-63..-1, 1..63} to each W entry
    out   = (x @ Wq + b1) @ Wq.T + b2

Algebraic restructure (exact reassociation):
    G = Wq @ Wq.T                (32x32)
    c = b1 @ Wq.T + b2           (32)
    out = x @ G + c

This removes the [N, 2048] intermediate entirely; the kernel becomes
DMA-bound (x in + out out = 2 MB/core).

Sharding: data-parallel over rows of x across 8 cores (8192 rows each).
Host-side layout choices (no on-device transposes needed):
  - x shard  -> xT4  [128, 2048]: 4 row-streams of 2048, feature dim on
               partitions (xT4[32b+f, n] = x[2048b+n, f]).
  - W        -> wT16 [128, 512]: W.T in 16 chunks of 128 H-rows
               (wT16[p, 32c+m] = W[m, 128c+p]).
  - b1       -> b1T  [128, 16]  (b1T[p, c] = b1[128c+p])
  - b2       -> b2b4 [128, 1]   (tiled 4x)
  - alpha_raw-> araw [128, 1]   (broadcast)
  - p4a      [32, 128] selection matrix (p4a[f, p] = [p%32 == f]) used to
               replicate G across the 4 partition groups via one matmul.

Device program per core:
  1. alpha path: ACT softplus + 1e-6, DVE reciprocal.
  2. quantize W on DVE/ACT: t = W/alpha; u = clip(round(|t|), 1, 63);
     Wq = u * sign(t)*alpha.  (round via +/- 2^23+2^22 magic)
  3. G|c: 16 accumulating PE matmuls lhsT=WqT_chunk, rhs=[WqT_chunk|b1_chunk]
     -> PSUM [32, 33]; broadcast to 4 partition groups with one matmul by p4a.
  4. main: 4 chunks of 512 columns; per chunk 4 diagonal-tile 32x32 matmuls
     (tile_position=(32b,32b)) compute out.T for the 4 row-streams, bias add
     fused into the PSUM->SBUF copy, DMA out.
"""

import sys

import numpy as np

sys.path.insert(0, "/opt/trn_rl_repo")

N, NF, H = 65536, 32, 2048
NCORES = 8
NLOC = N // NCORES  # 8192 rows per core
NS = NLOC // 4  # 2048 rows per stream
CHUNK = 512  # free-dim chunk = one PSUM bank of fp32
NCHUNK = NS // CHUNK  # 4
MAGIC = 12582912.0  # 2^23 + 2^22: fp32 round-to-nearest-even magic

_CACHE = {}


def build_nc():
    import concourse.bacc as bacc
    import concourse.mybir as mybir
    from concourse import tile

    fp32 = mybir.dt.float32
    Alu = mybir.AluOpType
    Act = mybir.ActivationFunctionType

    nc = bacc.Bacc("TRN2", target_bir_lowering=False, debug=False)
    xT4 = nc.declare_dram_parameter("xT4", [128, NS], fp32, isOutput=False)
    wT16 = nc.declare_dram_parameter("wT16", [128, 512], fp32, isOutput=False)
    b1T = nc.declare_dram_parameter("b1T", [128, 16], fp32, isOutput=False)
    b2b4 = nc.declare_dram_parameter("b2b4", [128, 1], fp32, isOutput=False)
    alv = nc.declare_dram_parameter("alv", [128, 1], fp32, isOutput=False)
    p4a = nc.declare_dram_parameter("p4a", [32, 128], fp32, isOutput=False)
    outT4 = nc.declare_dram_parameter("outT4", [128, NS], fp32, isOutput=True)

    with tile.TileContext(nc) as tc:
        with (
            tc.tile_pool(name="cpool", bufs=1) as cpool,
            tc.tile_pool(name="xpool", bufs=3) as xpool,
            tc.tile_pool(name="opool", bufs=3) as opool,
            tc.tile_pool(name="pso", bufs=2, space="PSUM") as pso,
            tc.tile_pool(name="psg", bufs=1, space="PSUM") as psg,
        ):
            # ---- alpha (host-computed softplus(alpha_raw)+1e-6, broadcast) ----
            # The gen3 ACT tables carry no Softplus/Ln entry, so the scalar
            # alpha is evaluated on host (jax-cpu, bit-identical to the
            # reference) and shipped pre-broadcast as [128,1].
            al1 = cpool.tile([128, 1], fp32)  # alpha
            nc.sync.dma_start(out=al1[:], in_=alv[:])
            inva = cpool.tile([128, 1], fp32)  # 1/alpha
            nc.vector.reciprocal(inva[:], al1[:])

            # ---- load W (chunked-transposed) and b1 ----
            wv = cpool.tile([128, 512], fp32)
            nc.sync.dma_start(out=wv[:], in_=wT16[:])
            # Wq tile interleaved as 16 blocks of [32 wq cols | 1 b1 col | 1 pad]
            # so each G-matmul's rhs [WqT_c | b1_c] is one contiguous AP.
            wq = cpool.tile([128, 544], fp32)
            wq3 = wq[:].rearrange("p (c u) -> p c u", u=34)
            nc.sync.dma_start(
                out=wq3[:, :, 32:33], in_=b1T[:].rearrange("p (c u) -> p c u", u=1)
            )

            # ---- quantize: Wq = clip(round(|W/alpha|),1,63) * sign(t) * alpha ----
            t_t = cpool.tile([128, 512], fp32)
            nc.any.tensor_scalar(t_t[:], wv[:], inva[:], None, Alu.mult)
            aab = cpool.tile([128, 512], fp32)  # |t| (ACT: abs_max is not a
            nc.scalar.activation(aab[:], t_t[:], Act.Abs)  # valid TS alu op)
            u_t = cpool.tile([128, 512], fp32)
            nc.any.tensor_scalar(u_t[:], aab[:], MAGIC, MAGIC, Alu.add, Alu.subtract)
            u2 = cpool.tile([128, 512], fp32)
            nc.any.tensor_scalar(u2[:], u_t[:], 1.0, 63.0, Alu.max, Alu.min)
            sg = cpool.tile([128, 512], fp32)  # sign(t) in {-1, 0, +1}
            nc.scalar.activation(sg[:], t_t[:], Act.Sign)
            v_t = cpool.tile([128, 512], fp32)  # signed level index
            nc.any.tensor_tensor(v_t[:], u2[:], sg[:], Alu.mult)
            nc.any.tensor_scalar(
                wq3[:, :, 0:32],
                v_t[:].rearrange("p (c u) -> p c u", u=32),
                al1[:],
                None,
                Alu.mult,
            )

            # ---- G | c: accumulate 16 chunk matmuls into PSUM [32, 33] ----
            ps_gc = psg.tile([32, 33], fp32)
            for c in range(16):
                nc.tensor.matmul(
                    ps_gc[:, :],
                    wq[:, 34 * c : 34 * c + 32],
                    wq[:, 34 * c : 34 * c + 33],
                    start=(c == 0),
                    stop=(c == 15),
                )
            gc_sb = cpool.tile([32, 33], fp32)
            nc.any.tensor_copy(gc_sb[:], ps_gc[:])

            # replicate [G | c] across the 4 partition groups: p4a.T @ gc
            p4_sb = cpool.tile([32, 128], fp32)
            nc.sync.dma_start(out=p4_sb[:], in_=p4a[:])
            ps_g4 = psg.tile([128, 33], fp32)
            nc.tensor.matmul(ps_g4[:, :], p4_sb[:], gc_sb[:], start=True, stop=True)
            g4_sb = cpool.tile([128, 32], fp32)
            nc.any.tensor_copy(g4_sb[:], ps_g4[:, 0:32])
            b2_sb = cpool.tile([128, 1], fp32)
            nc.sync.dma_start(out=b2_sb[:], in_=b2b4[:])
            cb_sb = cpool.tile([128, 1], fp32)  # bias row c+b2, per partition
            nc.any.tensor_scalar(cb_sb[:], ps_g4[:, 32:33], b2_sb[:], None, Alu.add)

            # ---- main pass: out.T chunks via 4 diagonal 32x32 PE tiles ----
            for ci in range(NCHUNK):
                x_sb = xpool.tile([128, CHUNK], fp32)
                nc.sync.dma_start(
                    out=x_sb[:], in_=xT4[:, CHUNK * ci : CHUNK * (ci + 1)]
                )
                ps_o = pso.tile([128, CHUNK], fp32)
                for b in range(4):
                    nc.tensor.matmul(
                        ps_o[32 * b : 32 * b + 32, :],
                        g4_sb[32 * b : 32 * b + 32, :],
                        x_sb[32 * b : 32 * b + 32, :],
                        start=True,
                        stop=True,
                        tile_position=(32 * b, 32 * b),
                    )
                o_sb = opool.tile([128, CHUNK], fp32)
                nc.any.tensor_scalar(o_sb[:], ps_o[:], cb_sb[:], None, Alu.add)
                nc.sync.dma_start(
                    out=outT4[:, CHUNK * ci : CHUNK * (ci + 1)], in_=o_sb[:]
                )

    nc.compile()
    return nc


def _alpha_of(alpha_raw):
    """softplus(alpha_raw[0]) + 1e-6 in fp32, computed exactly as the
    reference does (jax on cpu) — the gen3 ACT tables have no softplus."""
    import jax
    import jax.numpy as jnp

    with jax.default_device(jax.devices("cpu")[0]):
        a = jax.nn.softplus(jnp.asarray(alpha_raw, jnp.float32).reshape(-1)[0]) + 1e-6
        return np.float32(a)


def prep_in_maps(x, W, b1, b2, alpha_raw):
    x = np.ascontiguousarray(np.asarray(x, dtype=np.float32))
    W = np.asarray(W, dtype=np.float32)
    b1 = np.asarray(b1, dtype=np.float32).reshape(H)
    b2 = np.asarray(b2, dtype=np.float32).reshape(NF)

    wT16 = np.ascontiguousarray(
        W.T.reshape(16, 128, NF).transpose(1, 0, 2).reshape(128, 512)
    )
    b1T = np.ascontiguousarray(b1.reshape(16, 128).T)
    b2b4 = np.ascontiguousarray(np.tile(b2, 4).reshape(128, 1))
    alv = np.full((128, 1), _alpha_of(alpha_raw), dtype=np.float32)
    p4a = np.zeros((32, 128), dtype=np.float32)
    p4a[np.arange(128) % 32, np.arange(128)] = 1.0

    shared = dict(wT16=wT16, b1T=b1T, b2b4=b2b4, alv=alv, p4a=p4a)
    in_maps = []
    for i in range(NCORES):
        xs = x[i * NLOC : (i + 1) * NLOC]
        xT4 = np.ascontiguousarray(
            xs.reshape(4, NS, NF).transpose(0, 2, 1).reshape(128, NS)
        )
        in_maps.append({**shared, "xT4": xT4})
    return in_maps


def assemble_output(results):
    out = np.empty((N, NF), dtype=np.float32)
    for i, r in enumerate(results):
        oT4 = np.asarray(r["outT4"])
        out[i * NLOC : (i + 1) * NLOC] = (
            oT4.reshape(4, NF, NS).transpose(0, 2, 1).reshape(NLOC, NF)
        )
    return out


def kernel(x, W, b1, b2, alpha_raw):
    from concourse.bass_utils import run_bass_kernel_spmd

    if "nc" not in _CACHE:
        _CACHE["nc"] = build_nc()
    nc = _CACHE["nc"]
    in_maps = prep_in_maps(x, W, b1, b2, alpha_raw)
    res = run_bass_kernel_spmd(nc, in_maps, list(range(NCORES)))
    return assemble_output(res.results)


# revision 19
# speedup vs baseline: 1.1064x; 1.1064x over previous
"""Trainium2 Bass kernel for nn_MergerSingleW (vq_codebook).

Reference math:
    alpha = softplus(alpha_raw[0]) + 1e-6
    Wq    = nearest level in alpha*{-63..-1, 1..63} to each W entry
    out   = (x @ Wq + b1) @ Wq.T + b2

Algebraic restructure (exact reassociation):
    V = clip(round(|W|/alpha), 1, 63) * sign(W)     (integer levels)
    G = alpha^2 * (V @ V.T)                          (32x32; V@V.T is exact
                                                      integer arithmetic in fp32)
    c = alpha * (V @ b1) + b2                        (32)
    out = x @ G + c

This removes the [N, 2048] intermediate entirely; the kernel is DMA-bound
(x in + out out = 2 MB/core).

Sharding: data-parallel over rows of x across 8 cores (8192 rows each).
Host-side layout choices (no on-device transposes needed):
  - x shard  -> xT4  [128, 2048]: 4 row-streams of 2048, feature dim on
               partitions (xT4[32b+f, n] = x[2048b+n, f]).
  - kin      [128, 530]: consolidated constants — cols 0:512 = W.T in 16
               chunks of 128 H-rows (kin[p, 32c+m] = W[m, 128c+p]),
               cols 512:528 = b1 chunks (kin[p, 512+c] = b1[128c+p]),
               col 528 = b2 tiled 4x, col 529 = alpha (host softplus —
               the gen3 ACT tables have no Softplus entry).
  - p4a      [32, 128] selection matrix (p4a[f, p] = [p%32 == f]) used to
               replicate [G | c] across the 4 partition groups via one matmul.

Device program per core:
  1. x loads first on the Sync HWDGE ring (2 x 512 KB); constants on the
     Scalar HWDGE ring.  ACT-table pre-warm overlaps the DMAs.
  2. quantize W -> V: sg=Sign(W) and a=Abs(W*(1/alpha)) on ACT, round via
     +/-(2^23+2^22) magic and clamp(1,63) on DVE, V = u*sg.
  3. [G|c] raw: 16 accumulating PE matmuls lhsT=V_chunk, rhs=[V_chunk|b1_chunk]
     -> PSUM [32, 33]; scaled by alpha^2 / alpha on the PSUM->SBUF copies;
     5 tiny matmuls against the p4s selection constant expand it to the
     BLOCK-DIAGONAL Gbd [128, 128] (stream b's G in block (b,b), zeros
     elsewhere) plus the replicated bias column.
  4. main: 4 chunks of 512 columns; per chunk ONE full-array K=128 matmul
     (lhsT=Gbd) computes out.T for all 4 row-streams at once — fp32r
     single-pass needs full col_grp, which this satisfies; bias fused into
     the PSUM->SBUF copy on DVE, two 512 KB output DMAs on the Scalar ring.
"""

import sys

import numpy as np

sys.path.insert(0, "/opt/trn_rl_repo")

N, NF, H = 65536, 32, 2048
NCORES = 8
NLOC = N // NCORES  # 8192 rows per core
NS = NLOC // 4  # 2048 rows per stream
CHUNK = 512  # matmul moving-dim chunk = one PSUM bank of fp32
MAGIC = 12582912.0  # 2^23 + 2^22: fp32 round-to-nearest-even magic

USE_FP32R = True  # single-pass fp32 matmuls for the main pass (4x PE rate)

_CACHE = {}


def build_nc(use_fp32r=USE_FP32R):
    import concourse.bacc as bacc
    import concourse.mybir as mybir
    from concourse import tile

    fp32 = mybir.dt.float32
    fp32r = mybir.dt.float32r
    Alu = mybir.AluOpType
    Act = mybir.ActivationFunctionType

    # fp32r = raw single-pass fp32 through the PE (1 cyc/row at N>=256 vs 4
    # for two-pass fp32). The BIR verifier requires fp32r matmul operands to
    # be produced as fp32r, so the x input and the G tile are declared fp32r
    # natively (identical 4-byte layout).
    xdt = fp32r if use_fp32r else fp32

    nc = bacc.Bacc("TRN2", target_bir_lowering=False, debug=False)
    xT4 = nc.declare_dram_parameter("xT4", [128, NS], xdt, isOutput=False)
    kin = nc.declare_dram_parameter("kin", [128, 530], fp32, isOutput=False)
    p4s = nc.declare_dram_parameter("p4s", [32, 512], fp32, isOutput=False)
    outT4 = nc.declare_dram_parameter("outT4", [128, NS], fp32, isOutput=True)

    with tile.TileContext(nc) as tc:
        with (
            tc.tile_pool(name="cpool", bufs=1) as cpool,
            tc.tile_pool(name="pso", bufs=2, space="PSUM") as pso,
            tc.tile_pool(name="psg", bufs=1, space="PSUM") as psg,
        ):
            # ---- x loads first: 2 x 512KB on the Sync HWDGE ring ----
            x_lo = cpool.tile([128, 1024], xdt)
            x_hi = cpool.tile([128, 1024], xdt)
            nc.sync.dma_start(out=x_lo[:], in_=xT4[:, 0:1024])
            nc.sync.dma_start(out=x_hi[:], in_=xT4[:, 1024:2048])

            # ---- ACT table pre-warm (overlaps the DMAs) ----
            warm = cpool.tile([1, 1], fp32)
            nc.gpsimd.memset(warm[:], 0.0)
            warm2 = cpool.tile([1, 1], fp32)
            nc.scalar.activation(warm2[:], warm[:], Act.Abs)

            # ---- constants on the Scalar HWDGE ring ----
            ksb = cpool.tile([128, 530], fp32)
            nc.scalar.dma_start(out=ksb[:], in_=kin[:])
            p4_sb = cpool.tile([32, 512], fp32)
            nc.scalar.dma_start(out=p4_sb[:], in_=p4s[:])

            wv = ksb[:, 0:512]
            b1v = ksb[:, 512:528]
            b2v = ksb[:, 528:529]
            al1 = ksb[:, 529:530]

            # ---- quantize W -> V (integer levels, sign applied) ----
            inva = cpool.tile([128, 1], fp32)
            nc.vector.reciprocal(inva[:], al1)
            alsq = cpool.tile([128, 1], fp32)  # alpha^2
            nc.vector.tensor_tensor(alsq[:], al1, al1, Alu.mult)
            sg = cpool.tile([128, 512], fp32)  # sign(W) in {-1, 0, +1}
            nc.scalar.activation(sg[:], wv, Act.Sign)
            aab = cpool.tile([128, 512], fp32)  # |W| / alpha
            nc.scalar.activation(aab[:], wv, Act.Abs, scale=inva[:])
            u_t = cpool.tile([128, 512], fp32)  # round to nearest int
            nc.vector.tensor_scalar(u_t[:], aab[:], MAGIC, MAGIC, Alu.add, Alu.subtract)
            u2 = cpool.tile([128, 512], fp32)  # clamp to [1, 63]
            nc.vector.tensor_scalar(u2[:], u_t[:], 1.0, 63.0, Alu.max, Alu.min)
            # V tile interleaved as 16 blocks of [32 v cols | 1 b1 col | 1 pad]
            # so each G-matmul rhs [V_c | b1_c] is one contiguous AP.
            wq = cpool.tile([128, 544], fp32)
            wq3 = wq[:].rearrange("p (c u) -> p c u", u=34)
            nc.vector.tensor_tensor(
                wq3[:, :, 0:32],
                u2[:].rearrange("p (c u) -> p c u", u=32),
                sg[:].rearrange("p (c u) -> p c u", u=32),
                Alu.mult,
            )
            nc.vector.tensor_copy(
                wq3[:, :, 32:33], b1v.rearrange("p (c u) -> p c u", u=1)
            )

            # ---- [G | c] raw: accumulate 16 chunk matmuls into PSUM [32,33] ----
            ps_gc = psg.tile([32, 33], fp32)
            for c in range(16):
                nc.tensor.matmul(
                    ps_gc[:, :],
                    wq[:, 34 * c : 34 * c + 32],
                    wq[:, 34 * c : 34 * c + 33],
                    start=(c == 0),
                    stop=(c == 15),
                )
            # scale: G block by alpha^2, bias column by alpha (exact-integer
            # V@V.T only picks up one rounding here)
            gc_g = cpool.tile([32, 32], fp32)
            nc.vector.tensor_scalar(
                gc_g[:], ps_gc[:, 0:32], alsq[0:32, :], None, Alu.mult
            )
            gc_c = cpool.tile([32, 1], fp32)
            nc.vector.tensor_scalar(
                gc_c[:], ps_gc[:, 32:33], ksb[0:32, 529:530], None, Alu.mult
            )

            # expand to block-diagonal [Gbd | c4] via the p4s selectors:
            # matmul j writes G into partition block j / column block j only.
            ps_gbd = psg.tile([128, 129], fp32)
            for j in range(4):
                nc.tensor.matmul(
                    ps_gbd[:, 32 * j : 32 * j + 32],
                    p4_sb[:, 128 * j : 128 * j + 128],
                    gc_g[:],
                    start=True,
                    stop=True,
                )
            nc.tensor.matmul(
                ps_gbd[:, 128:129], p4_sb[:, 0:128], gc_c[:], start=True, stop=True
            )
            gbd = cpool.tile([128, 128], xdt)
            nc.vector.tensor_copy(gbd[:], ps_gbd[:, 0:128])
            cb_sb = cpool.tile([128, 1], fp32)  # bias row (+b2), per partition
            nc.vector.tensor_scalar(cb_sb[:], ps_gbd[:, 128:129], b2v, None, Alu.add)

            # ---- main pass: one full-array K=128 matmul per 512-chunk ----
            o_lo = cpool.tile([128, 1024], fp32)
            o_hi = cpool.tile([128, 1024], fp32)
            for ci in range(4):
                x_sb = (x_lo, x_hi)[ci // 2]
                o_sb = (o_lo, o_hi)[ci // 2]
                s = 512 * (ci % 2)
                ps_o = pso.tile([128, CHUNK], fp32)
                nc.tensor.matmul(
                    ps_o[:, :],
                    gbd[:],
                    x_sb[:, s : s + CHUNK],
                    start=True,
                    stop=True,
                )
                nc.vector.tensor_scalar(
                    o_sb[:, s : s + CHUNK], ps_o[:], cb_sb[:], None, Alu.add
                )
                if ci % 2 == 1:
                    nc.scalar.dma_start(
                        out=outT4[:, 1024 * (ci // 2) : 1024 * (ci // 2 + 1)],
                        in_=o_sb[:],
                    )

    nc.compile()
    return nc


def _alpha_of(alpha_raw):
    """softplus(alpha_raw[0]) + 1e-6 in fp32, computed exactly as the
    reference does (jax on cpu) — the gen3 ACT tables have no softplus."""
    import jax
    import jax.numpy as jnp

    with jax.default_device(jax.devices("cpu")[0]):
        a = jax.nn.softplus(jnp.asarray(alpha_raw, jnp.float32).reshape(-1)[0]) + 1e-6
        return np.float32(a)


def prep_in_maps(x, W, b1, b2, alpha_raw):
    x = np.ascontiguousarray(np.asarray(x, dtype=np.float32))
    W = np.asarray(W, dtype=np.float32)
    b1 = np.asarray(b1, dtype=np.float32).reshape(H)
    b2 = np.asarray(b2, dtype=np.float32).reshape(NF)

    kin = np.empty((128, 530), dtype=np.float32)
    kin[:, 0:512] = W.T.reshape(16, 128, NF).transpose(1, 0, 2).reshape(128, 512)
    kin[:, 512:528] = b1.reshape(16, 128).T
    kin[:, 528] = np.tile(b2, 4)
    kin[:, 529] = _alpha_of(alpha_raw)
    # p4s[f, 128j + p] = [p == 32j + f]: selector j scatters G's rows into
    # partition block j (zero elsewhere) -> block-diagonal expansion.
    p4s = np.zeros((32, 512), dtype=np.float32)
    for j in range(4):
        p4s[np.arange(32), 128 * j + 32 * j + np.arange(32)] = 1.0

    shared = dict(kin=kin, p4s=p4s)
    in_maps = []
    for i in range(NCORES):
        xs = x[i * NLOC : (i + 1) * NLOC]
        xT4 = np.ascontiguousarray(
            xs.reshape(4, NS, NF).transpose(0, 2, 1).reshape(128, NS)
        )
        in_maps.append({**shared, "xT4": xT4})
    return in_maps


def assemble_output(results):
    out = np.empty((N, NF), dtype=np.float32)
    for i, r in enumerate(results):
        oT4 = np.asarray(r["outT4"])
        out[i * NLOC : (i + 1) * NLOC] = (
            oT4.reshape(4, NF, NS).transpose(0, 2, 1).reshape(NLOC, NF)
        )
    return out


def kernel(x, W, b1, b2, alpha_raw):
    from concourse.bass_utils import run_bass_kernel_spmd

    if "nc" not in _CACHE:
        _CACHE["nc"] = build_nc()
    nc = _CACHE["nc"]
    in_maps = prep_in_maps(x, W, b1, b2, alpha_raw)
    res = run_bass_kernel_spmd(nc, in_maps, list(range(NCORES)))
    return assemble_output(res.results)


# revision 29
# speedup vs baseline: 1.2120x; 1.0955x over previous
"""Trainium2 Bass kernel for nn_MergerSingleW (vq_codebook).

Reference math:
    alpha = softplus(alpha_raw[0]) + 1e-6
    Wq    = nearest level in alpha*{-63..-1, 1..63} to each W entry
    out   = (x @ Wq + b1) @ Wq.T + b2

Algebraic restructure (exact reassociation):
    V = clip(round(|W|/alpha), 1, 63) * sign(W)     (integer levels)
    G = alpha^2 * (V @ V.T)                          (32x32; V@V.T is exact
                                                      integer arithmetic in fp32)
    c = alpha * (V @ b1) + b2                        (32)
    out = x @ G + c

This removes the [N, 2048] intermediate entirely; the kernel is DMA-bound
(x in + out out = 2 MB/core).

Sharding: data-parallel over rows of x across 8 cores (8192 rows each).
Host-side layout choices (no on-device transposes needed):
  - x shard  -> xT4  [128, 2048]: 4 row-streams of 2048, feature dim on
               partitions (xT4[32b+f, n] = x[2048b+n, f]).
  - kin      [128, 530]: consolidated constants — cols 0:512 = W.T in 16
               chunks of 128 H-rows (kin[p, 32c+m] = W[m, 128c+p]),
               cols 512:528 = b1 chunks (kin[p, 512+c] = b1[128c+p]),
               col 528 = b2 tiled 4x, col 529 = alpha (host softplus —
               the gen3 ACT tables have no Softplus entry).
  - p4a      [32, 128] selection matrix (p4a[f, p] = [p%32 == f]) used to
               replicate [G | c] across the 4 partition groups via one matmul.

Device program per core:
  1. x loads first on the Sync HWDGE ring (2 x 512 KB); constants on the
     Scalar HWDGE ring.  ACT-table pre-warm overlaps the DMAs.
  2. quantize W -> V: sg=Sign(W) and a=Abs(W*(1/alpha)) on ACT, round via
     +/-(2^23+2^22) magic and clamp(1,63) on DVE, V = u*sg.
  3. [G|c] raw: 16 accumulating PE matmuls lhsT=V_chunk, rhs=[V_chunk|b1_chunk]
     -> PSUM [32, 33]; scaled by alpha^2 / alpha on the PSUM->SBUF copies;
     5 tiny matmuls against the p4s selection constant expand it to the
     BLOCK-DIAGONAL Gbd [128, 128] (stream b's G in block (b,b), zeros
     elsewhere) plus the replicated bias column.
  4. main: 4 chunks of 512 columns; per chunk ONE full-array K=128 matmul
     (lhsT=Gbd) computes out.T for all 4 row-streams at once — fp32r
     single-pass needs full col_grp, which this satisfies; bias fused into
     the PSUM->SBUF copy on DVE, two 512 KB output DMAs on the Scalar ring.
"""

import sys

import numpy as np

sys.path.insert(0, "/opt/trn_rl_repo")

N, NF, H = 65536, 32, 2048
NCORES = 8
NLOC = N // NCORES  # 8192 rows per core
NS = NLOC // 4  # 2048 rows per stream
CHUNK = 512  # matmul moving-dim chunk = one PSUM bank of fp32
MAGIC = 12582912.0  # 2^23 + 2^22: fp32 round-to-nearest-even magic

USE_FP32R = True  # single-pass fp32 matmuls for the main pass (4x PE rate)

_CACHE = {}


def build_nc(use_fp32r=USE_FP32R):
    import concourse.bacc as bacc
    import concourse.mybir as mybir
    from concourse import tile

    fp32 = mybir.dt.float32
    fp32r = mybir.dt.float32r
    Alu = mybir.AluOpType
    Act = mybir.ActivationFunctionType

    # fp32r = raw single-pass fp32 through the PE (1 cyc/row at N>=256 vs 4
    # for two-pass fp32). The BIR verifier requires fp32r matmul operands to
    # be produced as fp32r, so the x input and the G tile are declared fp32r
    # natively (identical 4-byte layout).
    xdt = fp32r if use_fp32r else fp32

    nc = bacc.Bacc("TRN2", target_bir_lowering=False, debug=False)
    xT4 = nc.declare_dram_parameter("xT4", [128, NS], xdt, isOutput=False)
    kin = nc.declare_dram_parameter("kin", [128, 530], fp32, isOutput=False)
    p4a = nc.declare_dram_parameter("p4a", [32, 128], fp32, isOutput=False)
    gz = nc.declare_dram_parameter("gz", [128, 128], xdt, isOutput=False)
    outT4 = nc.declare_dram_parameter("outT4", [128, NS], fp32, isOutput=True)

    with tile.TileContext(nc) as tc:
        with (
            tc.tile_pool(name="cpool", bufs=1) as cpool,
            tc.tile_pool(name="pso", bufs=2, space="PSUM") as pso,
            tc.tile_pool(name="psg", bufs=1, space="PSUM") as psg,
        ):
            # ---- input DMAs in FIFO order on the Sync ring: the small
            # constants first (they gate the whole W-path), then x ----
            ksb = cpool.tile([128, 530], fp32)
            nc.sync.dma_start(out=ksb[:], in_=kin[:])
            p4_sb = cpool.tile([32, 128], fp32)
            nc.sync.dma_start(out=p4_sb[:], in_=p4a[:])
            gbd = cpool.tile([128, 128], xdt)  # zero-filled (memset can't
            nc.sync.dma_start(out=gbd[:], in_=gz[:])  # write fp32r)
            x_lo = cpool.tile([128, 1024], xdt)
            x_hi = cpool.tile([128, 1024], xdt)
            nc.sync.dma_start(out=x_lo[:], in_=xT4[:, 0:1024])
            nc.sync.dma_start(out=x_hi[:], in_=xT4[:, 1024:2048])

            # ---- ACT table pre-warm (overlaps the DMAs) ----
            warm = cpool.tile([1, 1], fp32)
            nc.gpsimd.memset(warm[:], 0.0)
            warm2 = cpool.tile([1, 1], fp32)
            nc.scalar.activation(warm2[:], warm[:], Act.Abs)

            wv = ksb[:, 0:512]
            b1v = ksb[:, 512:528]
            b2v = ksb[:, 528:529]
            al1 = ksb[:, 529:530]

            # ---- quantize W -> V (integer levels, sign applied) ----
            inva = cpool.tile([128, 1], fp32)
            nc.vector.reciprocal(inva[:], al1)
            alsq = cpool.tile([128, 1], fp32)  # alpha^2
            nc.vector.tensor_tensor(alsq[:], al1, al1, Alu.mult)
            aab = cpool.tile([128, 512], fp32)  # |W| / alpha (first: gates DVE)
            nc.scalar.activation(aab[:], wv, Act.Abs, scale=inva[:])
            sg = cpool.tile([128, 512], fp32)  # sign(W) in {-1, 0, +1}
            nc.scalar.activation(sg[:], wv, Act.Sign)
            u_t = cpool.tile([128, 512], fp32)  # round to nearest int
            nc.vector.tensor_scalar(u_t[:], aab[:], MAGIC, MAGIC, Alu.add, Alu.subtract)
            u2 = cpool.tile([128, 512], fp32)  # clamp to [1, 63]
            nc.vector.tensor_scalar(u2[:], u_t[:], 1.0, 63.0, Alu.max, Alu.min)
            # V tile interleaved as 16 blocks of [32 v cols | 1 b1 col | 1 pad]
            # so each G-matmul rhs [V_c | b1_c] is one contiguous AP.
            wq = cpool.tile([128, 544], fp32)
            wq3 = wq[:].rearrange("p (c u) -> p c u", u=34)
            nc.vector.tensor_tensor(
                wq3[:, :, 0:32],
                u2[:].rearrange("p (c u) -> p c u", u=32),
                sg[:].rearrange("p (c u) -> p c u", u=32),
                Alu.mult,
            )
            nc.vector.tensor_copy(
                wq3[:, :, 32:33], b1v.rearrange("p (c u) -> p c u", u=1)
            )

            # ---- [G | c] raw: accumulate 16 chunk matmuls into PSUM [32,33] ----
            ps_gc = psg.tile([32, 33], fp32)
            for c in range(16):
                nc.tensor.matmul(
                    ps_gc[:, :],
                    wq[:, 34 * c : 34 * c + 32],
                    wq[:, 34 * c : 34 * c + 33],
                    start=(c == 0),
                    stop=(c == 15),
                )
            gc_sb = cpool.tile([32, 33], fp32)
            nc.vector.tensor_copy(gc_sb[:], ps_gc[:])

            # replicate raw [G | c] across the 4 partition groups: p4a.T @ gc,
            # then 4 partition-aligned scaled copies build the block-diagonal
            # Gbd (zeros elsewhere kill the cross-stream terms), so the main
            # pass is ONE full-array K=128 matmul per chunk — fp32r's
            # "full col_grp only" restriction is satisfied.
            ps_g4 = psg.tile([128, 33], fp32)
            nc.tensor.matmul(ps_g4[:, :], p4_sb[:], gc_sb[:], start=True, stop=True)
            for b in range(4):
                nc.vector.tensor_scalar(
                    gbd[32 * b : 32 * b + 32, 32 * b : 32 * b + 32],
                    ps_g4[32 * b : 32 * b + 32, 0:32],
                    alsq[32 * b : 32 * b + 32, :],
                    None,
                    Alu.mult,
                )
            cb_sb = cpool.tile([128, 1], fp32)  # c = alpha * raw + b2
            nc.vector.tensor_scalar(
                cb_sb[:], ps_g4[:, 32:33], al1, b2v, Alu.mult, Alu.add
            )

            # ---- main pass: one full-array K=128 matmul per 512-chunk ----
            o_lo = cpool.tile([128, 1024], fp32)
            o_hi = cpool.tile([128, 1024], fp32)
            for ci in range(4):
                x_sb = (x_lo, x_hi)[ci // 2]
                o_sb = (o_lo, o_hi)[ci // 2]
                s = 512 * (ci % 2)
                ps_o = pso.tile([128, CHUNK], fp32)
                nc.tensor.matmul(
                    ps_o[:, :],
                    gbd[:],
                    x_sb[:, s : s + CHUNK],
                    start=True,
                    stop=True,
                )
                nc.vector.tensor_scalar(
                    o_sb[:, s : s + CHUNK], ps_o[:], cb_sb[:], None, Alu.add
                )
                # out DMAs on the Scalar ring: 1 MB as [0:1024], [1024:1536],
                # [1536:2048] so the final (completion-gating) DMA is small.
                if ci == 1:
                    nc.scalar.dma_start(out=outT4[:, 0:1024], in_=o_lo[:])
                elif ci >= 2:
                    s2 = 512 * ci
                    nc.scalar.dma_start(
                        out=outT4[:, s2 : s2 + 512], in_=o_hi[:, s : s + 512]
                    )

    nc.compile()
    return nc


def _alpha_of(alpha_raw):
    """softplus(alpha_raw[0]) + 1e-6 in fp32, computed exactly as the
    reference does (jax on cpu) — the gen3 ACT tables have no softplus."""
    import jax
    import jax.numpy as jnp

    with jax.default_device(jax.devices("cpu")[0]):
        a = jax.nn.softplus(jnp.asarray(alpha_raw, jnp.float32).reshape(-1)[0]) + 1e-6
        return np.float32(a)


def prep_in_maps(x, W, b1, b2, alpha_raw):
    x = np.ascontiguousarray(np.asarray(x, dtype=np.float32))
    W = np.asarray(W, dtype=np.float32)
    b1 = np.asarray(b1, dtype=np.float32).reshape(H)
    b2 = np.asarray(b2, dtype=np.float32).reshape(NF)

    kin = np.empty((128, 530), dtype=np.float32)
    kin[:, 0:512] = W.T.reshape(16, 128, NF).transpose(1, 0, 2).reshape(128, 512)
    kin[:, 512:528] = b1.reshape(16, 128).T
    kin[:, 528] = np.tile(b2, 4)
    kin[:, 529] = _alpha_of(alpha_raw)
    # p4a[f, p] = [p % 32 == f]: replicates [G | c] across partition groups.
    p4a = np.zeros((32, 128), dtype=np.float32)
    p4a[np.arange(128) % 32, np.arange(128)] = 1.0

    shared = dict(kin=kin, p4a=p4a, gz=np.zeros((128, 128), dtype=np.float32))
    in_maps = []
    for i in range(NCORES):
        xs = x[i * NLOC : (i + 1) * NLOC]
        xT4 = np.ascontiguousarray(
            xs.reshape(4, NS, NF).transpose(0, 2, 1).reshape(128, NS)
        )
        in_maps.append({**shared, "xT4": xT4})
    return in_maps


def assemble_output(results):
    out = np.empty((N, NF), dtype=np.float32)
    for i, r in enumerate(results):
        oT4 = np.asarray(r["outT4"])
        out[i * NLOC : (i + 1) * NLOC] = (
            oT4.reshape(4, NF, NS).transpose(0, 2, 1).reshape(NLOC, NF)
        )
    return out


def kernel(x, W, b1, b2, alpha_raw):
    from concourse.bass_utils import run_bass_kernel_spmd

    if "nc" not in _CACHE:
        _CACHE["nc"] = build_nc()
    nc = _CACHE["nc"]
    in_maps = prep_in_maps(x, W, b1, b2, alpha_raw)
    res = run_bass_kernel_spmd(nc, in_maps, list(range(NCORES)))
    return assemble_output(res.results)
